# revision 13
# baseline (speedup 1.0000x reference)
"""DeltaNet decode step on 8 Trainium2 NeuronCores (tensor-parallel over heads).

Contract: kernel(**inputs) takes the FULL unsharded inputs (numpy arrays,
same keys as the reference setup_inputs()) and returns the FULL output
[1, 4096, 1, 1] float32.

Sharding (8 cores, 16 heads -> 2 heads/core):
  - Wq/Wk rows, q/k conv weights+caches: 512 rows per core
  - Wv rows, v conv weights+caches, Wo columns: 1024 per core
  - state: 2 heads per core
  - output: each core computes a partial [4096] projection; host all-reduces.

Device kernel (memory-bound streaming, ~16.8MB/core):
  - Wq/Wk/Wv are streamed as single fp8-e4m3 (x128 scale) and consumed with
    DoubleRow matmuls: each [128, 2, 512] rhs carries TWO 128-row contraction
    chunks; lhsT carries the matching h chunks as fp8 (hi, lo) pairs in the
    M dim (hi = e4m3(16h), lo = e4m3(64*(16h - hi))), folded with scaled
    K=2 matmuls afterwards. End-to-end rel err ~1.7e-2 (gate 2e-2).
  - Wo is streamed in bf16 with ov cast to bf16 (error ~1e-3, negligible
    in quadrature).
  - The post-matvec chain (conv, l2norm, state update, combine) runs in
    128-lane column layout, injected into the Wv streaming phase.
"""

import sys
import types

sys.path.insert(0, "/opt/trn_rl_repo")

import numpy as np
import ml_dtypes

import concourse.bass as bass
import concourse.mybir as mybir
import concourse.tile as tile
from concourse import bacc
from concourse.bass_utils import run_bass_kernel_spmd

BF16 = ml_dtypes.bfloat16
E4 = ml_dtypes.float8_e4m3
F32 = mybir.dt.float32
BF = mybir.dt.bfloat16
F8 = mybir.dt.float8e4
AF = mybir.ActivationFunctionType
OP = mybir.AluOpType
PM = mybir.MatmulPerfMode

H = 4096
QK = 4096
VD = 8192
EPS = 1e-6
NCORES = 8
HPC = 2          # heads per core
RQ = 512         # q/k rows per core
RV = 1024        # v rows / Wo cols per core

SW = 128.0       # fp8 weight scale
SH_HI = 16.0     # fp8 h hi scale
SH_LO = 64.0     # fp8 h lo extra scale
# fold scales: q = row_hi/(SW*SH_HI) + row_lo/(SW*SH_HI*SH_LO)
FS_HI = 1.0 / (SW * SH_HI)
FS_LO = 1.0 / (SW * SH_HI * SH_LO)

_CACHE = {}


def _ensure_ntff_hook():
    """Install the axon NTFF profile hook shim (antenv.axon_hooks is absent
    in this image). Harmless if profiling is never requested."""
    if "antenv.axon_hooks" in sys.modules:
        return
    try:
        import antenv
        mod = types.ModuleType("antenv.axon_hooks")
        mod._hook = None
        mod.set_axon_ntff_profile_hook = lambda h: setattr(mod, "_hook", h)
        mod.get_axon_ntff_profile_hook = lambda: mod._hook
        sys.modules["antenv.axon_hooks"] = mod
        antenv.axon_hooks = mod
        from trn_agent_boot.trn_boot import _ntff_profile_via_ctypes
        mod._hook = _ntff_profile_via_ctypes("/opt/axon/libaxon_pjrt.so")
    except Exception:
        pass


def _build_nc():
    nc = bacc.Bacc(None)

    d = {}
    d["wqk8"] = nc.dram_tensor("wqk8", [4, 128, 8192], F8, kind="ExternalInput")
    d["wv8"] = nc.dram_tensor("wv8", [4, 128, 8192], F8, kind="ExternalInput")
    d["wo16"] = nc.dram_tensor("wo16", [4, 128, 8192], BF, kind="ExternalInput")
    d["hf8"] = nc.dram_tensor("hf8", [128, 64], F8, kind="ExternalInput")
    d["wab"] = nc.dram_tensor("wab", [128, 128], F32, kind="ExternalInput")
    d["hrep"] = nc.dram_tensor("hrep", [128, 128], F32, kind="ExternalInput")
    d["state_c"] = nc.dram_tensor("state_c", [128, 2048], F32, kind="ExternalInput")
    d["qkcache"] = nc.dram_tensor("qkcache", [128, 24], F32, kind="ExternalInput")
    d["qkconvw"] = nc.dram_tensor("qkconvw", [128, 32], F32, kind="ExternalInput")
    d["vcache"] = nc.dram_tensor("vcache", [128, 24], F32, kind="ExternalInput")
    d["vconvw"] = nc.dram_tensor("vconvw", [128, 32], F32, kind="ExternalInput")
    d["fsc"] = nc.dram_tensor("fsc", [2, 1], F32, kind="ExternalInput")
    out_d = nc.dram_tensor("out", [1, H], F32, kind="ExternalOutput")

    with tile.TileContext(nc) as tc:
        with (
            tc.tile_pool(name="smalls", bufs=1) as sm,
            tc.tile_pool(name="wp8", bufs=4) as wp8,
            tc.tile_pool(name="wpo", bufs=4) as wpo,
            tc.tile_pool(name="psum", bufs=8, space="PSUM") as pm,
        ):
            def emit():
                # ---- small input DMAs (SWDGE keeps the HWDGE ring clear) ----
                hf8 = sm.tile([128, 2, 32], F8, tag="hf8")
                nc.gpsimd.dma_start(
                    out=hf8[:], in_=d["hf8"][:].rearrange("p (i m) -> p i m", i=2))
                wab = sm.tile([128, 128], F32, tag="wab")
                hrep = sm.tile([128, 128], F32, tag="hrep")
                st = sm.tile([128, 2048], F32, tag="st")
                qkca = sm.tile([128, 24], F32, tag="qkca")
                qkcw = sm.tile([128, 32], F32, tag="qkcw")
                vca = sm.tile([128, 24], F32, tag="vca")
                vcw = sm.tile([128, 32], F32, tag="vcw")
                for t, src in [(wab, "wab"), (hrep, "hrep"), (st, "state_c"),
                               (qkca, "qkcache"), (qkcw, "qkconvw"),
                               (vca, "vcache"), (vcw, "vconvw")]:
                    nc.gpsimd.dma_start(out=t[:], in_=d[src][:])
                ones = sm.tile([1, 128], F32, tag="ones")
                nc.vector.memset(ones[:], 1.0)
                onesc = sm.tile([128, 1], F32, tag="onesc")
                nc.vector.memset(onesc[:], 1.0)
                epst = sm.tile([1, 1], F32, tag="epst")
                nc.vector.memset(epst[:], EPS)
                # fold scale column [FS_HI, FS_LO]^T (host constant)
                fsc = sm.tile([2, 1], F32, tag="fsc")
                nc.gpsimd.dma_start(out=fsc[:], in_=d["fsc"][:])

                # ---- alpha/beta matvec via DVE (hrep[p, 4cc+j] = h[cc*128+p])
                abm = sm.tile([128, 128], F32, tag="abm")
                nc.vector.tensor_mul(abm[:], wab[:], hrep[:])
                abr = sm.tile([128, 4], F32, tag="abr")
                nc.vector.reduce_sum(
                    abr[:],
                    abm[:].rearrange("p (cc f) -> p f cc", f=4),
                    axis=mybir.AxisListType.X)
                ps_ab = pm.tile([1, 4], F32, tag="ps", name="ps_ab")
                nc.tensor.matmul(ps_ab[0:1, :], onesc[:, 0:1], abr[:],
                                 start=True, stop=True)
                ab = sm.tile([1, 4], F32, tag="ab")
                nc.scalar.activation(ab[:], ps_ab[:], AF.Sigmoid)

                # ---- fp8 DoubleRow streaming matvec helper ----
                def stream8(dram, ps0, ps1, inject=None):
                    """dram [4, 128, 8192] fp8, layout (d, p, (pr two rh r)).
                    Accumulates rh=0 -> ps0[2,512], rh=1 -> ps1[2,512] with
                    M=2 (h hi, lo). inject: {d: callable} emitted after d."""
                    for dd in range(4):
                        t = wp8.tile([128, 8192], F8, tag="w8", name="w8t")
                        nc.sync.dma_start(out=t[:], in_=dram[dd])
                        tv = t[:].rearrange(
                            "p (pr two rh r) -> p pr two rh r",
                            pr=4, two=2, r=512)
                        for pr in range(4):
                            pair = 4 * dd + pr
                            lh = hf8[:, 0:2, 2 * pair:2 * pair + 2]
                            nc.tensor.matmul(
                                ps0[0:2, :], lh, tv[:, pr, 0:2, 0, :],
                                start=(pair == 0), stop=(pair == 15),
                                perf_mode=PM.DoubleRow)
                            nc.tensor.matmul(
                                ps1[0:2, :], lh, tv[:, pr, 0:2, 1, :],
                                start=(pair == 0), stop=(pair == 15),
                                perf_mode=PM.DoubleRow)
                        if inject and dd in inject:
                            inject[dd]()

                # psum tiles for q/k matvec (allocated before the chain's
                # psum tiles to keep pool-slot reuse hazard-free)
                ps_q = pm.tile([2, 512], F32, tag="ps", name="ps_q")
                ps_k = pm.tile([2, 512], F32, tag="ps", name="ps_k")
                qsb = sm.tile([2, 512], F32, tag="qsb")
                ksb = sm.tile([2, 512], F32, tag="ksb")

                # Rest of the q/k chain in 128-lane column layout
                # (cols 0-3 = k chunks, 4-7 = q chunks). Injected into the
                # Wv streaming phase to fill DMA-wait gaps.
                t_qk = pm.tile([128, 8], F32, tag="ps", name="t_qk")
                qkcol = sm.tile([128, 8], F32, tag="qkcol")
                qacc = sm.tile([128, 8], F32, tag="qacc")
                qtmp = sm.tile([128, 8], F32, tag="qtmp")
                x1 = sm.tile([128, 8], F32, tag="x1")
                sq = sm.tile([128, 8], F32, tag="sq")
                ps_ss = pm.tile([1, 8], F32, tag="ps", name="ps_ss")
                ssr = sm.tile([1, 8], F32, tag="ssr")
                ssh = sm.tile([1, 4], F32, tag="ssh")
                srt = sm.tile([1, 4], F32, tag="srt")
                rin = sm.tile([1, 4], F32, tag="rin")
                t_rn = pm.tile([128, 4], F32, tag="ps", name="t_rn")
                rbc = sm.tile([128, 4], F32, tag="rbc")
                qkn = sm.tile([128, 8], F32, tag="qkn")
                dm = sm.tile([128, 4], F32, tag="dm")
                ps_dot = pm.tile([1, 4], F32, tag="ps", name="ps_dot")
                dotr = sm.tile([1, 4], F32, tag="dotr")
                dot = sm.tile([1, 2], F32, tag="dot")
                bd = sm.tile([1, 2], F32, tag="bd")
                t_bc = pm.tile([128, 16], F32, tag="ps", name="t_bc")
                abig = sm.tile([128, 8], F32, tag="abig")
                bdbig = sm.tile([128, 8], F32, tag="bdbig")
                ps_stc = pm.tile([128, 16], F32, tag="ps", name="ps_stc")
                vacc = sm.tile([128, 8], F32, tag="vacc")
                vtmp = sm.tile([128, 8], F32, tag="vtmp")

                def chain_pre_v():
                    # v-conv cache taps (independent of the v matvec)
                    nc.vector.tensor_mul(vacc[:], vca[:, 0:8], vcw[:, 0:8])
                    for tpi in (1, 2):
                        nc.vector.tensor_mul(vtmp[:], vca[:, 8 * tpi:8 * tpi + 8],
                                             vcw[:, 8 * tpi:8 * tpi + 8])
                        nc.vector.tensor_add(vacc[:], vacc[:], vtmp[:])

                def chain_pe_0():
                    # scaled hi/lo fold + row->column (K=2 outer products)
                    for c in range(4):
                        nc.tensor.matmul(t_qk[:, c:c + 1],
                                         ksb[0:2, 128 * c:128 * c + 128],
                                         fsc[0:2, 0:1], start=True, stop=True)
                        nc.tensor.matmul(t_qk[:, 4 + c:5 + c],
                                         qsb[0:2, 128 * c:128 * c + 128],
                                         fsc[0:2, 0:1], start=True, stop=True)
                    nc.vector.tensor_copy(qkcol[:], t_qk[:])
                    # conv + silu in columns
                    nc.vector.tensor_mul(qacc[:], qkca[:, 0:8], qkcw[:, 0:8])
                    for tpi in (1, 2):
                        nc.vector.tensor_mul(qtmp[:], qkca[:, 8 * tpi:8 * tpi + 8],
                                             qkcw[:, 8 * tpi:8 * tpi + 8])
                        nc.vector.tensor_add(qacc[:], qacc[:], qtmp[:])
                    nc.vector.tensor_mul(qtmp[:], qkcol[:], qkcw[:, 24:32])
                    nc.vector.tensor_add(qacc[:], qacc[:], qtmp[:])
                    nc.scalar.activation(x1[:], qacc[:], AF.Sigmoid)
                    nc.vector.tensor_mul(x1[:], qacc[:], x1[:])
                    nc.vector.tensor_mul(sq[:], x1[:], x1[:])

                def chain_pe_1():
                    # per-column sum of squares, then per-head l2 scale
                    nc.tensor.matmul(ps_ss[0:1, :], onesc[:, 0:1], sq[:],
                                     start=True, stop=True)
                    nc.vector.tensor_copy(ssr[:], ps_ss[0:1, :])
                    nc.vector.reduce_sum(
                        ssh[0:1, 0:4],
                        ssr[0:1, :].rearrange("a (g t) -> a g t", t=2),
                        axis=mybir.AxisListType.X)
                    nc.scalar.activation(srt[:], ssh[:], AF.Sqrt,
                                         bias=epst[0:1, 0:1])
                    nc.vector.reciprocal(rin[:], srt[:])

                def chain_pe_2_pre():
                    # broadcast 1/norm, normalize columns
                    for j in range(4):
                        nc.tensor.matmul(t_rn[:, j:j + 1], ones[0:1, :],
                                         rin[0:1, j:j + 1], start=True, stop=True)
                    nc.vector.tensor_copy(rbc[:], t_rn[:])
                    for g in range(4):  # k_h0, k_h1, q_h0, q_h1 col pairs
                        nc.vector.tensor_scalar(
                            out=qkn[:, 2 * g:2 * g + 2],
                            in0=x1[:, 2 * g:2 * g + 2],
                            scalar1=rbc[:, g:g + 1], scalar2=None, op0=OP.mult)
                    # q.k dot per head
                    nc.vector.tensor_mul(dm[:], qkn[:, 4:8], qkn[:, 0:4])
                    nc.tensor.matmul(ps_dot[0:1, :], onesc[:, 0:1], dm[:],
                                     start=True, stop=True)
                    nc.vector.tensor_copy(dotr[:], ps_dot[0:1, :])
                    nc.vector.reduce_sum(
                        dot[0:1, 0:2],
                        dotr[0:1, :].rearrange("a (g t) -> a g t", t=2),
                        axis=mybir.AxisListType.X)
                    nc.vector.tensor_mul(bd[:], ab[0:1, 2:4], dot[0:1, 0:2])
                    # broadcast alpha / beta*dot to [128, 8] (4 cols per head)
                    for hh in range(HPC):
                        for vc in range(4):
                            nc.tensor.matmul(t_bc[:, 4 * hh + vc:4 * hh + vc + 1],
                                             ones[0:1, :], ab[0:1, hh:hh + 1],
                                             start=True, stop=True)
                            nc.tensor.matmul(t_bc[:, 8 + 4 * hh + vc:9 + 4 * hh + vc],
                                             ones[0:1, :], bd[0:1, hh:hh + 1],
                                             start=True, stop=True)
                    nc.vector.tensor_copy(abig[:], t_bc[:, 0:8])
                    nc.scalar.copy(bdbig[:], t_bc[:, 8:16])
                    # state matvecs (fp32, column outputs)
                    for hh in range(HPC):
                        for which in range(2):  # 0 -> k, 1 -> q
                            for vc in range(4):
                                col = 8 * which + 4 * hh + vc
                                for d2 in range(2):
                                    blk = 2 * hh + d2
                                    nc.tensor.matmul(
                                        ps_stc[:, col:col + 1],
                                        st[:, 512 * blk + 128 * vc:
                                           512 * blk + 128 * vc + 128],
                                        qkn[:, 4 * which + 2 * hh + d2:
                                            4 * which + 2 * hh + d2 + 1],
                                        start=(d2 == 0), stop=(d2 == 1))

                # ---- q/k matvec (packed rhs: rh=0 -> q rows, rh=1 -> k) ----
                stream8(d["wqk8"], ps_q, ps_k, inject={2: chain_pre_v})
                nc.vector.tensor_copy(qsb[:], ps_q[0:2, :])
                nc.scalar.copy(ksb[:], ps_k[0:2, :])

                # ---- v matvec (rh halves -> 2 psum tiles) ----
                ps_v0 = pm.tile([2, 512], F32, tag="ps", name="ps_v0")
                ps_v1 = pm.tile([2, 512], F32, tag="ps", name="ps_v1")
                stream8(d["wv8"], ps_v0, ps_v1,
                        inject={0: chain_pe_0, 1: chain_pe_1, 2: chain_pe_2_pre})
                vsb = sm.tile([2, 1024], F32, tag="vsb")
                nc.vector.tensor_copy(vsb[0:2, 0:512], ps_v0[0:2, :])
                nc.scalar.copy(vsb[0:2, 512:1024], ps_v1[0:2, :])
                # scaled K=2 fold+transpose: vcol[p,j] = fs.vsb[:,128j+p]
                t_v = pm.tile([128, 8], F32, tag="ps", name="t_v")
                for j in range(8):
                    nc.tensor.matmul(t_v[:, j:j + 1],
                                     vsb[0:2, 128 * j:128 * j + 128],
                                     fsc[0:2, 0:1], start=True, stop=True)
                vcol = sm.tile([128, 8], F32, tag="vcol")
                nc.vector.tensor_copy(vcol[:], t_v[:])

                # ---- v conv tap3 + silu (cache taps precomputed in vacc) ----
                nc.vector.tensor_mul(vtmp[:], vcol[:], vcw[:, 24:32])
                nc.vector.tensor_add(vtmp[:], vacc[:], vtmp[:])
                v1c = sm.tile([128, 8], F32, tag="v1c")
                nc.scalar.activation(v1c[:], vtmp[:], AF.Sigmoid)
                nc.vector.tensor_mul(v1c[:], vtmp[:], v1c[:])

                # ---- combine 8-wide: ov = a*qs + (b*dot)*(v - a*ks) ----
                ovc = sm.tile([128, 8], F32, tag="ovc")
                errc = sm.tile([128, 8], F32, tag="errc")
                t1c = sm.tile([128, 8], F32, tag="t1c")
                nc.vector.tensor_mul(errc[:], abig[:], ps_stc[:, 0:8])
                nc.vector.tensor_sub(errc[:], v1c[:], errc[:])
                nc.vector.tensor_mul(t1c[:], abig[:], ps_stc[:, 8:16])
                nc.vector.tensor_mul(errc[:], bdbig[:], errc[:])
                nc.vector.tensor_add(ovc[:], t1c[:], errc[:])

                # ---- ov -> bf16 (single) ----
                ov16 = sm.tile([128, 8], BF, tag="ov16")
                nc.vector.tensor_copy(ov16[:], ovc[:])

                # ---- output projection (bf16, M=1) ----
                ps_o = [pm.tile([1, 512], F32, tag="ps", name=f"ps_o{i}")
                        for i in range(8)]
                out_sb = sm.tile([1, H], F32, tag="out_sb")
                for dd in range(4):
                    t = wpo.tile([128, 8192], BF, tag="wo", name="wot")
                    nc.sync.dma_start(
                        out=t[:].rearrange("p (i r) -> p i r", i=2),
                        in_=d["wo16"][dd])
                    for i in range(2):
                        j = 2 * dd + i
                        for it in range(8):
                            nc.tensor.matmul(
                                ps_o[it][0:1, :], ov16[:, j:j + 1],
                                t[:, 4096 * i + 512 * it:4096 * i + 512 * it + 512],
                                start=(j == 0), stop=(j == 7))
                for it in range(8):
                    dst = out_sb[0:1, 512 * it:512 * it + 512]
                    if it % 2 == 0:
                        nc.vector.tensor_copy(dst, ps_o[it][0:1, :])
                    else:
                        nc.scalar.copy(dst, ps_o[it][0:1, :])
                nc.sync.dma_start(out=out_d[:], in_=out_sb[:])

            emit()

    nc.finalize()
    return nc


def _prep_in_maps(inputs):
    f32 = np.float32
    hid = np.asarray(inputs["hidden_states"], f32)[0, :, 0, 0]     # [4096]
    Wq = np.asarray(inputs["Wq"], f32)
    Wk = np.asarray(inputs["Wk"], f32)
    Wv = np.asarray(inputs["Wv"], f32)
    Wo = np.asarray(inputs["Wo"], f32)
    Wa = np.asarray(inputs["Wa"], f32)
    Wb = np.asarray(inputs["Wb"], f32)
    qcw = np.asarray(inputs["q_conv_w"], f32)[0]                   # [QK, 4]
    kcw = np.asarray(inputs["k_conv_w"], f32)[0]
    vcw = np.asarray(inputs["v_conv_w"], f32)[0]                   # [VD, 4]
    qca = np.asarray(inputs["q_cache"], f32)[0]                    # [QK, 3]
    kca = np.asarray(inputs["k_cache"], f32)[0]
    vca = np.asarray(inputs["v_cache"], f32)[0]                    # [VD, 3]
    state = np.asarray(inputs["state"], f32)[0]                    # [16,256,512]

    # h fp8 hi/lo pair, [128, 64]: col (two*32 + pair*2 + m),
    # value row j = (pair*2 + two)*128 + p
    h_hi8 = (hid * SH_HI).astype(E4)
    h_lo8 = ((hid * SH_HI - h_hi8.astype(f32)) * SH_LO).astype(E4)
    hp = np.stack([h_hi8, h_lo8], -1).reshape(16, 2, 128, 2)  # pair,two,p,m
    hf8 = np.ascontiguousarray(
        hp.transpose(2, 1, 0, 3).reshape(128, 64))            # p,(two pair m)

    # h replicated x4 for the DVE alpha/beta matvec: hrep[p, 4cc+j] = h[cc*128+p]
    hrep = np.ascontiguousarray(
        np.repeat(hid.reshape(32, 128).T[:, :, None], 4, axis=2).reshape(128, 128))

    def pack8(wt):
        """wt [4096, 1024] fp8 (contraction-major) -> [4, 128, 8192] with
        tile layout (d, p, (pr two rh r))."""
        a = wt.reshape(4, 4, 2, 128, 2, 512)      # d pr two p rh r
        return np.ascontiguousarray(
            a.transpose(0, 3, 1, 2, 4, 5).reshape(4, 128, 8192))

    in_maps = []
    for c in range(NCORES):
        rq = slice(c * RQ, (c + 1) * RQ)
        rv = slice(c * RV, (c + 1) * RV)
        wqk = np.concatenate([Wq[rq], Wk[rq]], axis=0)             # [1024, 4096]
        wqk8 = pack8(np.ascontiguousarray((wqk.T * SW)).astype(E4))
        wv8 = pack8(np.ascontiguousarray((Wv[rv].T * SW)).astype(E4))
        # Wo columns rv, transposed [1024, 4096] bf16, tiles (d, p, (i r))
        wot = np.ascontiguousarray(Wo[:, rv].T).astype(BF16)
        wo16 = np.ascontiguousarray(
            wot.reshape(4, 2, 128, 4096).transpose(0, 2, 1, 3).reshape(4, 128, 8192))

        wab = np.concatenate([Wa[2 * c:2 * c + 2], Wb[2 * c:2 * c + 2]], 0)
        wab_sb = np.ascontiguousarray(
            wab.reshape(4, 32, 128).transpose(2, 1, 0).reshape(128, 128))
        st_sb = np.ascontiguousarray(
            state[2 * c:2 * c + 2].reshape(2, 2, 128, 512)
            .transpose(2, 0, 1, 3).reshape(128, 2048))

        # q/k conv in column layout [128, 8*taps]: per tap, cols 0-3 = k
        # chunks (k idx 128c+p), cols 4-7 = q chunks
        qk_ca = np.concatenate(
            [np.concatenate([kca[rq, t].reshape(4, 128).T,
                             qca[rq, t].reshape(4, 128).T], 1)
             for t in range(3)], 1)
        qk_cw = np.concatenate(
            [np.concatenate([kcw[rq, t].reshape(4, 128).T,
                             qcw[rq, t].reshape(4, 128).T], 1)
             for t in range(4)], 1)
        # v conv in column layout [128, 8*taps]: vcol[p, 8t+cc] = v[128cc+p, t]
        v_ca = np.ascontiguousarray(
            vca[rv].reshape(8, 128, 3).transpose(1, 2, 0).reshape(128, 24))
        v_cw = np.ascontiguousarray(
            vcw[rv].reshape(8, 128, 4).transpose(1, 2, 0).reshape(128, 32))

        in_maps.append({
            "wqk8": wqk8, "wv8": wv8, "wo16": wo16,
            "hf8": hf8, "wab": wab_sb, "hrep": hrep, "state_c": st_sb,
            "qkcache": np.ascontiguousarray(qk_ca),
            "qkconvw": np.ascontiguousarray(qk_cw),
            "vcache": v_ca, "vconvw": v_cw,
            "fsc": np.array([[FS_HI], [FS_LO]], f32),
        })
    return in_maps


def _run(inputs, trace=False, tmpdir=None):
    _ensure_ntff_hook()
    if "nc" not in _CACHE:
        _CACHE["nc"] = _build_nc()
    nc = _CACHE["nc"]
    in_maps = _prep_in_maps(inputs)
    res = run_bass_kernel_spmd(nc, in_maps, list(range(NCORES)),
                               trace=trace, tmpdir=tmpdir)
    acc = np.zeros(H, np.float64)
    for c in range(NCORES):
        acc += res.results[c]["out"][0].astype(np.float64)
    out = acc.astype(np.float32).reshape(1, H, 1, 1)
    return out, res


def kernel(**inputs):
    out, _ = _run(inputs, trace=False)
    return out


def kernel_traced(tmpdir=None, **inputs):
    return _run(inputs, trace=True, tmpdir=tmpdir)


# revision 16
# speedup vs baseline: 1.4185x; 1.4185x over previous
"""DeltaNet decode step on 8 Trainium2 NeuronCores (tensor-parallel over heads).

Contract: kernel(**inputs) takes the FULL unsharded inputs (numpy arrays,
same keys as the reference setup_inputs()) and returns the FULL output
[1, 4096, 1, 1] float32.

Sharding (8 cores, 16 heads -> 2 heads/core):
  - Wq/Wk rows, q/k conv weights+caches: 512 rows per core
  - Wv rows, v conv weights+caches, Wo columns: 1024 per core
  - state: 2 heads per core
  - output: each core computes a partial [4096] projection; host all-reduces.

Device kernel (memory-bound streaming, ~16.4MB/core):
  - Wq/Wk/Wv stream as single fp8-e4m3 (x128 scale), consumed by DoubleRow
    matmuls: each [128, 2, 512] rhs carries TWO 128-row contraction chunks;
    lhsT carries the matching h chunks as fp8 (hi, lo) pairs in the M dim
    (hi = e4m3(16h), lo = e4m3(64*(16h - hi))), folded by scaled K=2
    matmuls into columns. End-to-end rel err ~1.7e-2 (gate 2e-2).
  - Wo streams in bf16 with ov cast to bf16.
  - l2-normalization of q/k heads is deferred: the state matvecs run on
    raw silu(conv()) vectors as 4 batched bf16 [2,512] row matmuls
    (lhsT = (k,q) column pairs), and the 1/||.|| factors fold into the
    per-head combine scalars (a*rk, a*rq, b*dot*rq*rk).
  - ACT table loads are hoisted off the critical path (Silu preloaded via
    a dummy op before the post-stream silu(v)).
"""

import sys
import types

sys.path.insert(0, "/opt/trn_rl_repo")

import numpy as np
import ml_dtypes

import concourse.bass as bass
import concourse.mybir as mybir
import concourse.tile as tile
from concourse import bacc
from concourse.bass_utils import run_bass_kernel_spmd

BF16 = ml_dtypes.bfloat16
E4 = ml_dtypes.float8_e4m3
F32 = mybir.dt.float32
BF = mybir.dt.bfloat16
F8 = mybir.dt.float8e4
AF = mybir.ActivationFunctionType
OP = mybir.AluOpType
PM = mybir.MatmulPerfMode

H = 4096
QK = 4096
VD = 8192
EPS = 1e-6
NCORES = 8
HPC = 2          # heads per core
RQ = 512         # q/k rows per core
RV = 1024        # v rows / Wo cols per core

SW = 128.0       # fp8 weight scale
SH_HI = 16.0     # fp8 h hi scale
SH_LO = 64.0     # fp8 h lo extra scale
# fold scales: x = row_hi/(SW*SH_HI) + row_lo/(SW*SH_HI*SH_LO); both are
# powers of two -> exact in bf16
FS_HI = 1.0 / (SW * SH_HI)
FS_LO = 1.0 / (SW * SH_HI * SH_LO)

_CACHE = {}


def _ensure_ntff_hook():
    """Install the axon NTFF profile hook shim (antenv.axon_hooks is absent
    in this image). Harmless if profiling is never requested."""
    if "antenv.axon_hooks" in sys.modules:
        return
    try:
        import antenv
        mod = types.ModuleType("antenv.axon_hooks")
        mod._hook = None
        mod.set_axon_ntff_profile_hook = lambda h: setattr(mod, "_hook", h)
        mod.get_axon_ntff_profile_hook = lambda: mod._hook
        sys.modules["antenv.axon_hooks"] = mod
        antenv.axon_hooks = mod
        from trn_agent_boot.trn_boot import _ntff_profile_via_ctypes
        mod._hook = _ntff_profile_via_ctypes("/opt/axon/libaxon_pjrt.so")
    except Exception:
        pass


def _build_nc():
    nc = bacc.Bacc(None)

    d = {}
    d["wqk8"] = nc.dram_tensor("wqk8", [4, 128, 8192], F8, kind="ExternalInput")
    d["wv8"] = nc.dram_tensor("wv8", [4, 128, 8192], F8, kind="ExternalInput")
    d["wo16"] = nc.dram_tensor("wo16", [4, 128, 8192], BF, kind="ExternalInput")
    d["hf8"] = nc.dram_tensor("hf8", [128, 64], F8, kind="ExternalInput")
    d["wab"] = nc.dram_tensor("wab", [128, 128], F32, kind="ExternalInput")
    d["hrep"] = nc.dram_tensor("hrep", [128, 128], F32, kind="ExternalInput")
    d["state16"] = nc.dram_tensor("state16", [128, 2048], BF, kind="ExternalInput")
    d["qkcache"] = nc.dram_tensor("qkcache", [128, 24], F32, kind="ExternalInput")
    d["qkconvw"] = nc.dram_tensor("qkconvw", [128, 32], F32, kind="ExternalInput")
    d["vcache"] = nc.dram_tensor("vcache", [128, 24], F32, kind="ExternalInput")
    d["vconvw"] = nc.dram_tensor("vconvw", [128, 32], F32, kind="ExternalInput")
    d["fsc16"] = nc.dram_tensor("fsc16", [2, 1], BF, kind="ExternalInput")
    d["eye16"] = nc.dram_tensor("eye16", [2, 2], BF, kind="ExternalInput")
    out_d = nc.dram_tensor("out", [1, H], F32, kind="ExternalOutput")

    with tile.TileContext(nc) as tc:
        with (
            tc.tile_pool(name="smalls", bufs=1) as sm,
            tc.tile_pool(name="wp8", bufs=4) as wp8,
            tc.tile_pool(name="wpo", bufs=4) as wpo,
            tc.tile_pool(name="psum", bufs=8, space="PSUM") as pm,
        ):
            def emit():
                # ---- small input DMAs (SWDGE keeps the HWDGE ring clear) ----
                hf8 = sm.tile([128, 2, 32], F8, tag="hf8")
                nc.gpsimd.dma_start(
                    out=hf8[:], in_=d["hf8"][:].rearrange("p (i m) -> p i m", i=2))
                wab = sm.tile([128, 128], F32, tag="wab")
                hrep = sm.tile([128, 128], F32, tag="hrep")
                st16 = sm.tile([128, 2048], BF, tag="st16")
                qkca = sm.tile([128, 24], F32, tag="qkca")
                qkcw = sm.tile([128, 32], F32, tag="qkcw")
                vca = sm.tile([128, 24], F32, tag="vca")
                vcw = sm.tile([128, 32], F32, tag="vcw")
                fsc = sm.tile([2, 1], BF, tag="fsc")
                eye = sm.tile([2, 2], BF, tag="eye")
                for t, src in [(wab, "wab"), (hrep, "hrep"), (fsc, "fsc16"),
                               (eye, "eye16"), (st16, "state16"),
                               (qkca, "qkcache"), (qkcw, "qkconvw"),
                               (vca, "vcache"), (vcw, "vconvw")]:
                    nc.gpsimd.dma_start(out=t[:], in_=d[src][:])
                ones = sm.tile([1, 128], F32, tag="ones")
                nc.vector.memset(ones[:], 1.0)
                onesc = sm.tile([128, 1], F32, tag="onesc")
                nc.vector.memset(onesc[:], 1.0)
                epst = sm.tile([1, 1], F32, tag="epst")
                nc.vector.memset(epst[:], EPS)

                # ---- psum tiles (allocation order fixes pool-slot reuse) ----
                ps_ab = pm.tile([1, 4], F32, tag="ps", name="ps_ab")
                ps_q = pm.tile([2, 512], F32, tag="ps", name="ps_q")
                ps_k = pm.tile([2, 512], F32, tag="ps", name="ps_k")
                ps_v0 = pm.tile([2, 512], F32, tag="ps", name="ps_v0")
                ps_v1 = pm.tile([2, 512], F32, tag="ps", name="ps_v1")
                t_col = pm.tile([128, 8], F32, tag="ps", name="t_col")
                t_row = pm.tile([1, 8], F32, tag="ps", name="t_row")
                ps_st0 = pm.tile([2, 512], F32, tag="ps", name="ps_st0")
                ps_st1 = pm.tile([2, 512], F32, tag="ps", name="ps_st1")
                ps_stc = pm.tile([128, 16], F32, tag="ps", name="ps_stc")

                # ---- sbuf chain tiles ----
                ab = sm.tile([1, 4], F32, tag="ab")
                qsb16 = sm.tile([2, 512], BF, tag="qsb16")
                ksb16 = sm.tile([2, 512], BF, tag="ksb16")
                qkcol = sm.tile([128, 8], F32, tag="qkcol")
                qacc = sm.tile([128, 8], F32, tag="qacc")
                qtmp = sm.tile([128, 8], F32, tag="qtmp")
                x1 = sm.tile([128, 8], F32, tag="x1")
                x116 = sm.tile([128, 8], BF, tag="x116")
                sq = sm.tile([128, 8], F32, tag="sq")
                ssr = sm.tile([1, 8], F32, tag="ssr")
                ssh = sm.tile([1, 4], F32, tag="ssh")
                rin = sm.tile([1, 4], F32, tag="rin")
                dm = sm.tile([128, 4], F32, tag="dm")
                dotr = sm.tile([1, 4], F32, tag="dotr")
                dot = sm.tile([1, 2], F32, tag="dot")
                akq = sm.tile([1, 4], F32, tag="akq")   # [ak0 ak1 aq0 aq1]
                bd = sm.tile([1, 2], F32, tag="bd")
                abc6 = sm.tile([128, 6], F32, tag="abc6")
                vacc = sm.tile([128, 8], F32, tag="vacc")
                vtmp = sm.tile([128, 8], F32, tag="vtmp")
                stsb0 = sm.tile([2, 512], BF, tag="stsb0")
                stsb1 = sm.tile([2, 512], BF, tag="stsb1")
                vsb16 = sm.tile([2, 1024], BF, tag="vsb16")
                vcol = sm.tile([128, 8], F32, tag="vcol")
                v1c = sm.tile([128, 8], F32, tag="v1c")
                ovc = sm.tile([128, 8], F32, tag="ovc")
                errc = sm.tile([128, 4], F32, tag="errc")
                t1c = sm.tile([128, 4], F32, tag="t1c")
                ov16 = sm.tile([128, 8], BF, tag="ov16")
                dum = sm.tile([1, 1], F32, tag="dum")
                out_sb = sm.tile([1, H], F32, tag="out_sb")

                # ---- injected work: alpha/beta + conv cache taps ----
                def pre_ab():
                    # hrep[p, 4cc+j] = h[cc*128+p]; wab[p, 4cc+j] = Wab[j, ...]
                    abm = sm.tile([128, 128], F32, tag="abm")
                    nc.vector.tensor_mul(abm[:], wab[:], hrep[:])
                    abr = sm.tile([128, 4], F32, tag="abr")
                    nc.vector.reduce_sum(
                        abr[:],
                        abm[:].rearrange("p (cc f) -> p f cc", f=4),
                        axis=mybir.AxisListType.X)
                    nc.tensor.matmul(ps_ab[0:1, :], onesc[:, 0:1], abr[:],
                                     start=True, stop=True)
                    nc.scalar.activation(ab[:], ps_ab[:], AF.Sigmoid)

                def pre_taps():
                    # q/k conv cache taps -> qacc; v conv cache taps -> vacc
                    nc.vector.tensor_mul(qacc[:], qkca[:, 0:8], qkcw[:, 0:8])
                    for tpi in (1, 2):
                        nc.vector.tensor_mul(qtmp[:], qkca[:, 8 * tpi:8 * tpi + 8],
                                             qkcw[:, 8 * tpi:8 * tpi + 8])
                        nc.vector.tensor_add(qacc[:], qacc[:], qtmp[:])
                    nc.vector.tensor_mul(vacc[:], vca[:, 0:8], vcw[:, 0:8])
                    for tpi in (1, 2):
                        nc.vector.tensor_mul(vtmp[:], vca[:, 8 * tpi:8 * tpi + 8],
                                             vcw[:, 8 * tpi:8 * tpi + 8])
                        nc.vector.tensor_add(vacc[:], vacc[:], vtmp[:])

                # ---- fp8 DoubleRow streaming matvec ----
                def stream8(dram, ps0, ps1, inject=None):
                    """dram [4, 128, 8192] fp8, layout (d, p, (pr two rh r)).
                    rh=0 -> ps0[2,512], rh=1 -> ps1[2,512], M=2 (h hi, lo)."""
                    for dd in range(4):
                        t = wp8.tile([128, 8192], F8, tag="w8", name="w8t")
                        nc.sync.dma_start(out=t[:], in_=dram[dd])
                        tv = t[:].rearrange(
                            "p (pr two rh r) -> p pr two rh r",
                            pr=4, two=2, r=512)
                        for pr in range(4):
                            pair = 4 * dd + pr
                            lh = hf8[:, 0:2, 2 * pair:2 * pair + 2]
                            nc.tensor.matmul(
                                ps0[0:2, :], lh, tv[:, pr, 0:2, 0, :],
                                start=(pair == 0), stop=(pair == 15),
                                perf_mode=PM.DoubleRow)
                            nc.tensor.matmul(
                                ps1[0:2, :], lh, tv[:, pr, 0:2, 1, :],
                                start=(pair == 0), stop=(pair == 15),
                                perf_mode=PM.DoubleRow)
                        if inject and dd in inject:
                            inject[dd]()

                def chain_pe_0():
                    # scaled hi/lo fold + row->column (K=2, bf16)
                    for c in range(4):
                        nc.tensor.matmul(t_col[:, c:c + 1],
                                         ksb16[0:2, 128 * c:128 * c + 128],
                                         fsc[0:2, 0:1], start=True, stop=True)
                        nc.tensor.matmul(t_col[:, 4 + c:5 + c],
                                         qsb16[0:2, 128 * c:128 * c + 128],
                                         fsc[0:2, 0:1], start=True, stop=True)
                    nc.vector.tensor_copy(qkcol[:], t_col[:])
                    # conv tap3 + silu in columns (raw, un-normalized)
                    nc.vector.tensor_mul(qtmp[:], qkcol[:], qkcw[:, 24:32])
                    nc.vector.tensor_add(qtmp[:], qacc[:], qtmp[:])
                    nc.scalar.activation(x1[:], qtmp[:], AF.Silu)
                    nc.vector.tensor_copy(x116[:], x1[:])
                    nc.vector.tensor_mul(sq[:], x1[:], x1[:])

                def chain_pe_1():
                    # per-column sum of squares -> per-head 1/||.||
                    nc.tensor.matmul(t_row[0:1, :], onesc[:, 0:1], sq[:],
                                     start=True, stop=True)
                    nc.vector.tensor_copy(ssr[:], t_row[0:1, :])
                    nc.vector.reduce_sum(
                        ssh[0:1, 0:4],
                        ssr[0:1, :].rearrange("a (g t) -> a g t", t=2),
                        axis=mybir.AxisListType.X)
                    srt = sm.tile([1, 4], F32, tag="srt")
                    nc.scalar.activation(srt[:], ssh[:], AF.Sqrt,
                                         bias=epst[0:1, 0:1])
                    nc.vector.reciprocal(rin[:], srt[:])
                    # raw q.k dot per head
                    nc.vector.tensor_mul(dm[:], x1[:, 4:8], x1[:, 0:4])

                def chain_pe_2():
                    nc.tensor.matmul(t_row[0:1, 0:4], onesc[:, 0:1], dm[:],
                                     start=True, stop=True)
                    nc.vector.tensor_copy(dotr[:], t_row[0:1, 0:4])
                    nc.vector.reduce_sum(
                        dot[0:1, 0:2],
                        dotr[0:1, :].rearrange("a (g t) -> a g t", t=2),
                        axis=mybir.AxisListType.X)
                    # per-head scalars: ak = a*rk, aq = a*rq,
                    # bd = b * dot_raw * rk * rq
                    nc.vector.tensor_mul(akq[0:1, 0:2], ab[0:1, 0:2], rin[0:1, 0:2])
                    nc.vector.tensor_mul(akq[0:1, 2:4], ab[0:1, 0:2], rin[0:1, 2:4])
                    nc.vector.tensor_mul(dot[:], dot[:], rin[0:1, 0:2])
                    nc.vector.tensor_mul(dot[:], dot[:], rin[0:1, 2:4])
                    nc.vector.tensor_mul(bd[:], ab[0:1, 2:4], dot[:])
                    # silu table preload for the post-stream silu(v)
                    nc.scalar.activation(dum[:], epst[:], AF.Silu)
                    # broadcast [ak0 ak1 aq0 aq1 bd0 bd1] to 128 partitions
                    for j in range(4):
                        nc.tensor.matmul(t_col[:, j:j + 1], ones[0:1, :],
                                         akq[0:1, j:j + 1], start=True, stop=True)
                    for j in range(2):
                        nc.tensor.matmul(t_col[:, 4 + j:5 + j], ones[0:1, :],
                                         bd[0:1, j:j + 1], start=True, stop=True)
                    nc.vector.tensor_copy(abc6[:], t_col[:, 0:6])
                    # state matvecs, batched: lhsT = (k,q) column pairs of raw
                    # x1 (bf16), rhs = state rows -> ps_st[hh] rows (ks, qs)
                    xv = x116[:].rearrange("p (g c) -> p c g", c=4)
                    for hh in range(HPC):
                        pst = ps_st0 if hh == 0 else ps_st1
                        for d2 in range(2):
                            blk = 2 * hh + d2
                            nc.tensor.matmul(
                                pst[0:2, :], xv[:, 2 * hh + d2, 0:2],
                                st16[:, 512 * blk:512 * blk + 512],
                                start=(d2 == 0), stop=(d2 == 1))

                # ---- phase 1: q/k matvec (rh=0 -> q rows, rh=1 -> k rows) ----
                stream8(d["wqk8"], ps_q, ps_k,
                        inject={0: pre_ab, 1: pre_taps})
                nc.vector.tensor_copy(qsb16[:], ps_q[0:2, :])
                nc.vector.tensor_copy(ksb16[:], ps_k[0:2, :])

                # ---- phase 2: v matvec with injected chain ----
                stream8(d["wv8"], ps_v0, ps_v1,
                        inject={0: chain_pe_0, 1: chain_pe_1, 2: chain_pe_2})

                # ---- phase 3: post-stream chain ----
                nc.vector.tensor_copy(stsb0[:], ps_st0[0:2, :])
                nc.vector.tensor_copy(stsb1[:], ps_st1[0:2, :])
                nc.vector.tensor_copy(vsb16[0:2, 0:512], ps_v0[0:2, :])
                nc.vector.tensor_copy(vsb16[0:2, 512:1024], ps_v1[0:2, :])
                # state rows -> columns: ps_stc col (8hh + 2c + n), n=0 ks n=1 qs
                for hh in range(HPC):
                    ssb = stsb0 if hh == 0 else stsb1
                    for c in range(4):
                        nc.tensor.matmul(
                            ps_stc[:, 8 * hh + 2 * c:8 * hh + 2 * c + 2],
                            ssb[0:2, 128 * c:128 * c + 128],
                            eye[0:2, 0:2], start=True, stop=True)
                # v hi/lo fold to columns
                for j in range(8):
                    nc.tensor.matmul(t_col[:, j:j + 1],
                                     vsb16[0:2, 128 * j:128 * j + 128],
                                     fsc[0:2, 0:1], start=True, stop=True)
                nc.vector.tensor_copy(vcol[:], t_col[:])
                # v conv tap3 + silu
                nc.vector.tensor_mul(vtmp[:], vcol[:], vcw[:, 24:32])
                nc.vector.tensor_add(vtmp[:], vacc[:], vtmp[:])
                nc.scalar.activation(v1c[:], vtmp[:], AF.Silu)
                # combine: ov = aq*qs + bd*(v1 - ak*ks)   (per head)
                stc = ps_stc[:].rearrange("p (hh c n) -> p hh n c", hh=2, n=2)
                for hh in range(HPC):
                    nc.vector.tensor_scalar(out=errc[:], in0=stc[:, hh, 0, :],
                                            scalar1=abc6[:, hh:hh + 1],
                                            scalar2=None, op0=OP.mult)
                    nc.vector.tensor_sub(errc[:], v1c[:, 4 * hh:4 * hh + 4], errc[:])
                    nc.vector.tensor_scalar(out=t1c[:], in0=stc[:, hh, 1, :],
                                            scalar1=abc6[:, 2 + hh:3 + hh],
                                            scalar2=None, op0=OP.mult)
                    nc.vector.tensor_scalar(out=errc[:], in0=errc[:],
                                            scalar1=abc6[:, 4 + hh:5 + hh],
                                            scalar2=None, op0=OP.mult)
                    nc.vector.tensor_add(ovc[:, 4 * hh:4 * hh + 4], t1c[:], errc[:])
                nc.vector.tensor_copy(ov16[:], ovc[:])

                # ---- phase 4: output projection (bf16, M=1) ----
                ps_o = [pm.tile([1, 512], F32, tag="ps", name=f"ps_o{i}")
                        for i in range(8)]
                for dd in range(4):
                    t = wpo.tile([128, 8192], BF, tag="wo", name="wot")
                    nc.sync.dma_start(out=t[:], in_=d["wo16"][dd])
                    for i in range(2):
                        j = 2 * dd + i
                        for it in range(8):
                            nc.tensor.matmul(
                                ps_o[it][0:1, :], ov16[:, j:j + 1],
                                t[:, 4096 * i + 512 * it:4096 * i + 512 * it + 512],
                                start=(j == 0), stop=(j == 7))
                for it in range(8):
                    dst = out_sb[0:1, 512 * it:512 * it + 512]
                    if it % 2 == 0:
                        nc.vector.tensor_copy(dst, ps_o[it][0:1, :])
                    else:
                        nc.scalar.copy(dst, ps_o[it][0:1, :])
                nc.sync.dma_start(out=out_d[:], in_=out_sb[:])

            emit()

    nc.finalize()
    return nc


def _prep_in_maps(inputs):
    f32 = np.float32
    hid = np.asarray(inputs["hidden_states"], f32)[0, :, 0, 0]     # [4096]
    Wq = np.asarray(inputs["Wq"], f32)
    Wk = np.asarray(inputs["Wk"], f32)
    Wv = np.asarray(inputs["Wv"], f32)
    Wo = np.asarray(inputs["Wo"], f32)
    Wa = np.asarray(inputs["Wa"], f32)
    Wb = np.asarray(inputs["Wb"], f32)
    qcw = np.asarray(inputs["q_conv_w"], f32)[0]                   # [QK, 4]
    kcw = np.asarray(inputs["k_conv_w"], f32)[0]
    vcw = np.asarray(inputs["v_conv_w"], f32)[0]                   # [VD, 4]
    qca = np.asarray(inputs["q_cache"], f32)[0]                    # [QK, 3]
    kca = np.asarray(inputs["k_cache"], f32)[0]
    vca = np.asarray(inputs["v_cache"], f32)[0]                    # [VD, 3]
    state = np.asarray(inputs["state"], f32)[0]                    # [16,256,512]

    # h fp8 hi/lo pair, [128, 64]: col (two*32 + pair*2 + m),
    # value row j = (pair*2 + two)*128 + p
    h_hi8 = (hid * SH_HI).astype(E4)
    h_lo8 = ((hid * SH_HI - h_hi8.astype(f32)) * SH_LO).astype(E4)
    hp = np.stack([h_hi8, h_lo8], -1).reshape(16, 2, 128, 2)  # pair,two,p,m
    hf8 = np.ascontiguousarray(
        hp.transpose(2, 1, 0, 3).reshape(128, 64))            # p,(two pair m)

    # h replicated x4 for the DVE alpha/beta matvec: hrep[p, 4cc+j] = h[cc*128+p]
    hrep = np.ascontiguousarray(
        np.repeat(hid.reshape(32, 128).T[:, :, None], 4, axis=2).reshape(128, 128))

    def pack8(wt):
        """wt [4096, 1024] fp8 (contraction-major) -> [4, 128, 8192] with
        tile layout (d, p, (pr two rh r))."""
        a = wt.reshape(4, 4, 2, 128, 2, 512)      # d pr two p rh r
        return np.ascontiguousarray(
            a.transpose(0, 3, 1, 2, 4, 5).reshape(4, 128, 8192))

    in_maps = []
    for c in range(NCORES):
        rq = slice(c * RQ, (c + 1) * RQ)
        rv = slice(c * RV, (c + 1) * RV)
        wqk = np.concatenate([Wq[rq], Wk[rq]], axis=0)             # [1024, 4096]
        wqk8 = pack8(np.ascontiguousarray((wqk.T * SW)).astype(E4))
        wv8 = pack8(np.ascontiguousarray((Wv[rv].T * SW)).astype(E4))
        # Wo columns rv, transposed [1024, 4096] bf16, tiles (d, p, (i r))
        wot = np.ascontiguousarray(Wo[:, rv].T).astype(BF16)
        wo16 = np.ascontiguousarray(
            wot.reshape(4, 2, 128, 4096).transpose(0, 2, 1, 3).reshape(4, 128, 8192))

        wab = np.concatenate([Wa[2 * c:2 * c + 2], Wb[2 * c:2 * c + 2]], 0)
        wab_sb = np.ascontiguousarray(
            wab.reshape(4, 32, 128).transpose(2, 1, 0).reshape(128, 128))
        st_sb = np.ascontiguousarray(
            state[2 * c:2 * c + 2].reshape(2, 2, 128, 512)
            .transpose(2, 0, 1, 3).reshape(128, 2048)).astype(BF16)

        # q/k conv in column layout [128, 8*taps]: per tap, cols 0-3 = k
        # chunks (k idx 128c+p), cols 4-7 = q chunks
        qk_ca = np.concatenate(
            [np.concatenate([kca[rq, t].reshape(4, 128).T,
                             qca[rq, t].reshape(4, 128).T], 1)
             for t in range(3)], 1)
        qk_cw = np.concatenate(
            [np.concatenate([kcw[rq, t].reshape(4, 128).T,
                             qcw[rq, t].reshape(4, 128).T], 1)
             for t in range(4)], 1)
        # v conv in column layout [128, 8*taps]: vcol[p, 8t+cc] = v[128cc+p, t]
        v_ca = np.ascontiguousarray(
            vca[rv].reshape(8, 128, 3).transpose(1, 2, 0).reshape(128, 24))
        v_cw = np.ascontiguousarray(
            vcw[rv].reshape(8, 128, 4).transpose(1, 2, 0).reshape(128, 32))

        in_maps.append({
            "wqk8": wqk8, "wv8": wv8, "wo16": wo16,
            "hf8": hf8, "wab": wab_sb, "hrep": hrep, "state16": st_sb,
            "qkcache": np.ascontiguousarray(qk_ca),
            "qkconvw": np.ascontiguousarray(qk_cw),
            "vcache": v_ca, "vconvw": v_cw,
            "fsc16": np.array([[FS_HI], [FS_LO]], f32).astype(BF16),
            "eye16": np.eye(2, dtype=f32).astype(BF16),
        })
    return in_maps


def _run(inputs, trace=False, tmpdir=None):
    _ensure_ntff_hook()
    if "nc" not in _CACHE:
        _CACHE["nc"] = _build_nc()
    nc = _CACHE["nc"]
    in_maps = _prep_in_maps(inputs)
    res = run_bass_kernel_spmd(nc, in_maps, list(range(NCORES)),
                               trace=trace, tmpdir=tmpdir)
    acc = np.zeros(H, np.float64)
    for c in range(NCORES):
        acc += res.results[c]["out"][0].astype(np.float64)
    out = acc.astype(np.float32).reshape(1, H, 1, 1)
    return out, res


def kernel(**inputs):
    out, _ = _run(inputs, trace=False)
    return out


def kernel_traced(tmpdir=None, **inputs):
    return _run(inputs, trace=True, tmpdir=tmpdir)
